# revision 39
# baseline (speedup 1.0000x reference)
"""Trainium2 Bass kernel for: softmax((hidden@w1+b1) @ ((hidden+pre_emb)@w2+b2)^T).

Shapes: hidden/pre_emb [4, 4096, 1024], w1/w2 [1024,1024], b1/b2 [1024].
Output: [4, 4096, 4096] float32.

Sharding: 8 cores = 4 batches x 2 token-halves. Core c=(b,p) owns queries AND
keys [p*2048,(p+1)*2048) of batch b. Each core projects only its OWN 2048 keys
(iT = ((hidden+pre_emb)@w2+b2)^T, built strip-by-strip on the PE); the full
4096-key iT is assembled by pairwise DRAM AllGathers over replica groups
[[0,1],[2,3],[4,5],[6,7]]. This halves proj2 PE work vs projecting all keys
per core (826K -> ~390K fewer PE cycles overall). The exchange is pipelined:
one 2MB AllGather per 512-key strip, emitted one strip behind the compute so
collective latency hides under phase-1 compute + proj1.

Rounds (flash-softmax over 2 key halves, both read back from the AllGather
output at static offsets -- round r uses rank r's slots, so the instruction
stream is parity-uniform):
  proj1: all four 512-query chunks transposed + projected up front (keeps the
    PE fed while the collective completes); hT [1024e, 2048q] fp32r stays
    RESIDENT in SBUF for both rounds (no spill/reload).
  round 0 = keys 0:2048: per 128-query block, 4x [128,512] PSUM tiles of 8
    accumulating fp32r matmuls; DVE row-max and ACT exp(x-max) both read the
    PSUM directly (no SBUF copy hop; scores live in PSUM until exp, 6-buf
    pool). exp writes bf16 (+row-sum accum) -> unnormalized scratch in DRAM.
  round 1 = keys 2048:4096: same PSUM-direct scores + flash-style stat merge;
    fresh half scaled to bf16 and written to cols 2048:4096, scratch half
    rescaled into cols 0:2048. The output itself is bf16 (values in [0,1],
    host upcasts; ~0.2%% quantization vs the 2%% gate) which halves the
    largest DMA stream. Output writes alternate sync/gpsimd queues.

All matmuls fp32r (full-rate 1cyc/row at >=512 moving, ~12-bit mantissa;
output rel err ~1.9e-3). Transposes fp32r (1.5 vs 2.0 cyc/row for fp32 --
numerically free, downstream matmuls re-round anyway). fp32r flows end-to-end
from DRAM (inputs/weights declared float32r: same bits, BIR-verifier-clean,
no cast copies). Biases arrive host-transposed [128,8] so no
descriptor-heavy rearrange DMA sits at the head of the sync queue. SBUF is
managed as two allocation stacks (side="left"/"right") so pool lifetimes that
are not LIFO-nestable still fit: weights/loads/own-iT close left, round-0 iT
and score blocks rotate right. Per-core PE floor: 524K cyc scores + 131K
proj1 + 131K proj2 + ~49K transposes ~= 836K cyc ~= 348us @2.4GHz.
"""

import numpy as np

import concourse.bass as bass
import concourse.tile as tile
from concourse import bacc, masks, mybir
from concourse.bass_utils import run_bass_kernel_spmd

F32 = mybir.dt.float32
BF16 = mybir.dt.bfloat16
F32R = mybir.dt.float32r
I32 = mybir.dt.int32
AF = mybir.ActivationFunctionType
ALU = mybir.AluOpType

B, S, D = 4, 4096, 1024
QP = S // 2          # queries per core = 2048
N_CORES = 8
KH = S // 2          # keys per round = 2048

_cache = {}
TRACE = False
SIM_NO_CC = False   # replace the AllGather with local DMAs (TimelineSim only)
LAST_EXEC_NS = None


def _build():
    if "nc" in _cache:
        return _cache["nc"]

    nc = bacc.Bacc("TRN2", target_bir_lowering=False, debug=False,
                   enable_asserts=False, num_devices=N_CORES)

    hid_q = nc.dram_tensor("hid_q", [QP, D], F32R, kind="ExternalInput").ap()
    pre_own = nc.dram_tensor("pre_own", [QP, D], F32R, kind="ExternalInput").ap()
    w1_d = nc.dram_tensor("w1", [D, D], F32R, kind="ExternalInput").ap()
    w2_d = nc.dram_tensor("w2", [D, D], F32R, kind="ExternalInput").ap()
    b1_d = nc.dram_tensor("b1", [128, 8], F32, kind="ExternalInput").ap()
    b2_d = nc.dram_tensor("b2", [128, 8], F32, kind="ExternalInput").ap()
    out_d = nc.dram_tensor("out", [QP, S], BF16, kind="ExternalOutput").ap()

    from contextlib import ExitStack
    with tile.TileContext(nc) as tc:
        w1ctx = ExitStack()
        w2ctx = ExitStack()
        ldctx = ExitStack()
        it2ctx = ExitStack()
        fbctx = ExitStack()
        with tc.tile_pool(name="consts", bufs=1) as consts, \
             tc.tile_pool(name="ht", bufs=1) as htpool, \
             tc.tile_pool(name="scb", bufs=1) as scbpool, \
             tc.tile_pool(name="keep", bufs=1) as keep, \
             tc.tile_pool(name="st", bufs=1) as stpool, \
             tc.tile_pool(name="dram", bufs=1, space="DRAM") as dpool:

            loads = ldctx.enter_context(tc.tile_pool(name="loads", bufs=3))
            strips = ldctx.enter_context(tc.tile_pool(name="strips", bufs=1))
            w2pool = w2ctx.enter_context(tc.tile_pool(name="w2pool", bufs=1))
            prctx = ExitStack()
            pstr = prctx.enter_context(
                tc.tile_pool(name="pstr", bufs=3, space="PSUM", side="right"))
            ppr = prctx.enter_context(
                tc.tile_pool(name="ppr", bufs=2, space="PSUM", side="right"))

            ident = consts.tile([128, 128], F32)
            masks.make_identity(nc, ident[:])
            identr_t = consts.tile([128, 128], F32R)
            nc.vector.tensor_copy(identr_t[:], ident[:])
            identr = identr_t[:]
            # biases laid out [128, 8]: column mo = b[mo*128:(mo+1)*128]
            b1t = consts.tile([128, 8], F32)
            b2t = consts.tile([128, 8], F32)

            # peel the first key-chunk loads ahead of the weight transfers so
            # the PE's first transposes aren't queued behind 4MB of weights
            pre_lt = []
            for half in range(2):
                r0 = half * 256
                lt = loads.tile([128, 2 * D], F32R, tag="load", name=f"pre_lt{half}")
                for j in range(2):
                    nc.sync.dma_start(
                        lt[:, j * D:(j + 1) * D],
                        hid_q[r0 + j * 128:r0 + (j + 1) * 128, :])
                    nc.gpsimd.dma_start(
                        lt[:, j * D:(j + 1) * D],
                        pre_own[r0 + j * 128:r0 + (j + 1) * 128, :],
                        accum_op=ALU.add)
                pre_lt.append(lt)

            # weights: DMA straight into fp32r tiles (bitcast view, same bits)
            w2r = []
            for ki in range(8):
                wr = w2pool.tile([128, D], F32R, tag=f"w2_{ki}", name=f"wr2_{ki}")
                nc.sync.dma_start(wr[:], w2_d[ki * 128:(ki + 1) * 128, :])
                w2r.append(wr)
            # biases arrive host-transposed as [128, 8]: single cheap DMAs,
            # placed behind the weights on the sync queue
            nc.sync.dma_start(b2t[:], b2_d)
            nc.sync.dma_start(b1t[:], b1_d)

            scratch = dpool.tile([QP, KH], BF16)
            # collective bounce buffers, kt-major: slot [r, kt, mo] so the
            # first strips every score group needs (all mo at kt=0) are the
            # first bytes through the link and the readback
            cc_in = dpool.tile([4, 8, 128, 512], F32R)
            cc_outs = [dpool.tile([2, 8, 128, 512], F32R, name=f"cc_out{kt}")
                       for kt in range(4)]

            # per-(qc,qb) saved stats from round 0: cols [2*qi]=rowmax, [2*qi+1]=rowsum
            svt = keep.tile([128, 32], F32, name="svt", tag="svt")
            saved = [svt[:, 2 * i:2 * i + 2] for i in range(16)]

            act_copy = nc.scalar.copy
            vec_copy = nc.vector.tensor_copy

            def transpose_128(src_ap, dst_ap, eng):
                tp = pstr.tile([128, 128], F32R, tag="tr")
                nc.tensor.transpose(tp[:], src_ap, identr)
                eng(dst_ap, tp[:])

            # ---- phase 1: iT for OWN keys (only feeds the collective) ----
            itownctx = ExitStack()
            itownpool = itownctx.enter_context(tc.tile_pool(name="itown", bufs=1))
            iTo = [itownpool.tile([128, KH], F32R, tag=f"ito{mo}", name=f"ito_{mo}")
                   for mo in range(8)]
            def emit_exchange(kt):
                # exchange strip kt (own iT cols) with the pair peer; under
                # SIM_NO_CC the data movement is deferred to one block after
                # phase 1 (the real collective moves data on the NRT rings,
                # not this engine queue)
                for mo in range(8):
                    nc.gpsimd.dma_start(cc_in[kt, mo],
                                        iTo[mo][:, kt * 512:(kt + 1) * 512])
                if not SIM_NO_CC:
                    nc.gpsimd.collective_compute(
                        "AllGather", ALU.bypass,
                        replica_groups=[[0, 1], [2, 3], [4, 5], [6, 7]],
                        ins=[cc_in[kt].opt()], outs=[cc_outs[kt].opt()])

            for kt in range(4):           # 512-key strips
                if kt > 0:
                    # previous strip's exchange: emitted here so its queue
                    # waits are already satisfied and never block this
                    # strip's accumulate-loads
                    emit_exchange(kt - 1)
                sumT = [strips.tile([128, 512], F32R, tag=f"str{ki}",
                                    name=f"sumT_{kt}_{ki}")
                        for ki in range(8)]
                for half in range(2):     # 256-key load chunks
                    r0 = kt * 512 + half * 256
                    if kt == 0:
                        lt = pre_lt[half]
                    else:
                        lt = loads.tile([128, 2 * D], F32R, tag="load",
                                        name=f"lt{kt}_{half}")
                        nc.sync.dma_start(
                            lt[:], hid_q[r0:r0 + 256, :].rearrange(
                                "(j p) c -> p j c", p=128))
                        nc.gpsimd.dma_start(
                            lt[:], pre_own[r0:r0 + 256, :].rearrange(
                                "(j p) c -> p j c", p=128),
                            accum_op=ALU.add)
                    for j in range(2):
                        st = half * 2 + j
                        for ki in range(8):
                            transpose_128(
                                lt[:, j * D + ki * 128:j * D + (ki + 1) * 128],
                                sumT[ki][:, st * 128:(st + 1) * 128],
                                act_copy if ki % 2 == 0 else vec_copy)
                for mo in range(8):
                    ps = ppr.tile([128, 512], F32, tag="pr")
                    for ki in range(8):
                        nc.tensor.matmul(ps[:], w2r[ki][:, mo * 128:(mo + 1) * 128],
                                         sumT[ki][:], start=(ki == 0), stop=(ki == 7))
                    nc.scalar.activation(iTo[mo][:, kt * 512:(kt + 1) * 512], ps[:],
                                         AF.Identity, bias=b2t[:, mo:mo + 1])
            emit_exchange(3)
            if SIM_NO_CC:
                for kt in range(4):
                    for mo in range(8):
                        nc.gpsimd.dma_start(cc_outs[kt][0, mo], cc_in[kt, mo])
                        nc.gpsimd.dma_start(cc_outs[kt][1, mo], cc_in[kt, mo])
            itownctx.close()
            w2ctx.close()
            itactx = ExitStack()
            itpool = itactx.enter_context(
                tc.tile_pool(name="ita", bufs=1, side="right"))
            # round-0 iT = global keys 0:2048 (rank-0 slots), striped per
            # (mo,kt) so score matmuls can start as strips land
            iT = [itpool.tile([128, KH], F32R, tag=f"it{mo}", name=f"it0_{mo}")
                  for mo in range(8)]
            for kt in range(4):
                for mo in range(8):
                    nc.gpsimd.dma_start(iT[mo][:, kt * 512:(kt + 1) * 512],
                                        cc_outs[kt][0, mo])

            # ---- queries ----
            w1pool = w1ctx.enter_context(tc.tile_pool(name="w1pool", bufs=1))
            w1r = []
            for ki in range(8):
                wr = w1pool.tile([128, D], F32R, tag=f"w1_{ki}", name=f"wr1_{ki}")
                nc.sync.dma_start(wr[:], w1_d[ki * 128:(ki + 1) * 128, :])
                w1r.append(wr)

            # hT is resident for the whole kernel: [128, QP] per e-chunk,
            # built per 512-query chunk in round 0, reused as-is in round 1
            hTr = [htpool.tile([128, QP], F32R, tag=f"ht{mo}", name=f"hT_{mo}")
                   for mo in range(8)]

            def build_hT(rnd, qc):
                if rnd == 1:
                    return hTr
                hqT = [strips.tile([128, 512], F32R, tag=f"str{ki}",
                                   name=f"hqT{qc}_{ki}")
                       for ki in range(8)]
                for half in range(2):
                    r0 = qc * 512 + half * 256
                    hq = loads.tile([128, 2 * D], F32R, tag="load",
                                    name=f"hq{qc}_{half}")
                    nc.sync.dma_start(
                        hq[:], hid_q[r0:r0 + 256, :].rearrange(
                            "(j p) c -> p j c", p=128))
                    for j in range(2):
                        st = half * 2 + j
                        for ki in range(8):
                            transpose_128(
                                hq[:, j * D + ki * 128:j * D + (ki + 1) * 128],
                                hqT[ki][:, st * 128:(st + 1) * 128],
                                act_copy if ki % 2 == 0 else vec_copy)
                for mo in range(8):
                    ps = ppr.tile([128, 512], F32, tag="pr")
                    for ki in range(8):
                        nc.tensor.matmul(ps[:],
                                         w1r[ki][:, mo * 128:(mo + 1) * 128],
                                         hqT[ki][:], start=(ki == 0),
                                         stop=(ki == 7))
                    nc.scalar.activation(
                        hTr[mo][:, qc * 512:(qc + 1) * 512], ps[:],
                        AF.Identity, bias=b1t[:, mo:mo + 1])
                return hTr

            def scores_block(rnd, qc, hT, cur_iT, fbpool, sc2pool):
                for qb in range(4):       # 128-query blocks
                    qi = qc * 4 + qb
                    q0 = qc * 512 + qb * 128
                    blockmax = stpool.tile([128, 4], F32, tag="bm")
                    sums = stpool.tile([128, 4], F32, tag="sm")
                    if rnd == 1:
                        # prefetch the round-0 scratch halves for the fixup
                        # while this qb's scores are still on the PE
                        pre_fbb = []
                        for kh in range(2):
                            fbb = fbpool.tile([128, 1024], BF16, tag="fbb",
                                              name=f"fbb{qi}_{kh}")
                            nc.gpsimd.dma_start(
                                fbb[:],
                                scratch[q0:q0 + 128, kh * 1024:(kh + 1) * 1024])
                            pre_fbb.append(fbb)
                    # scores stay in PSUM until exp: no SBUF copy hop
                    pss = []
                    for kb in range(4):
                        ps = psc2.tile([128, 512], F32, tag="ps")
                        for mo in range(8):
                            nc.tensor.matmul(
                                ps[:], hT[mo][:, q0:q0 + 128],
                                cur_iT[mo][:, kb * 512:(kb + 1) * 512],
                                start=(mo == 0), stop=(mo == 7))
                        nc.vector.tensor_reduce(blockmax[:, kb:kb + 1], ps[:],
                                                axis=mybir.AxisListType.X,
                                                op=ALU.max)
                        pss.append(ps)
                    rowmax = stpool.tile([128, 1], F32, tag="rm")
                    nc.vector.tensor_reduce(rowmax[:], blockmax[:],
                                            axis=mybir.AxisListType.X, op=ALU.max)
                    negmax = stpool.tile([128, 1], F32, tag="nm")
                    nc.vector.tensor_scalar_mul(negmax[:], rowmax[:], -1.0)
                    # exp reads PSUM, writes bf16 (output precision anyway)
                    bbt = scbpool if rnd == 0 else sc2pool
                    bb = bbt.tile([128, 2048], BF16, tag="blkb",
                                  name=f"bb{rnd}_{qi}")
                    for kb in range(4):
                        nc.scalar.activation(bb[:, kb * 512:(kb + 1) * 512],
                                             pss[kb][:], AF.Exp, bias=negmax[:],
                                             accum_out=sums[:, kb:kb + 1])
                    rowsum = stpool.tile([128, 1], F32, tag="rs")
                    nc.vector.tensor_reduce(rowsum[:], sums[:],
                                            axis=mybir.AxisListType.X, op=ALU.add)
                    if rnd == 0:
                        nc.vector.tensor_copy(saved[qi][:, 0:1], rowmax[:])
                        nc.vector.tensor_copy(saved[qi][:, 1:2], rowsum[:])
                        for kh in range(2):
                            nc.gpsimd.dma_start(
                                scratch[q0:q0 + 128, kh * 1024:(kh + 1) * 1024],
                                bb[:, kh * 1024:(kh + 1) * 1024])
                    else:
                        m1 = saved[qi][:, 0:1]
                        s1 = saved[qi][:, 1:2]
                        # negm = -max(m1, rowmax)
                        negm = stpool.tile([128, 1], F32, tag="ngm")
                        nc.vector.tensor_scalar(negm[:], rowmax[:], m1, -1.0,
                                                op0=ALU.max, op1=ALU.mult)
                        e1 = stpool.tile([128, 1], F32, tag="e1")
                        nc.scalar.activation(e1[:], m1, AF.Exp, bias=negm[:])
                        e2 = stpool.tile([128, 1], F32, tag="e2")
                        nc.scalar.activation(e2[:], rowmax[:], AF.Exp, bias=negm[:])
                        # z = s2*e2 + (s1*e1)
                        t1 = stpool.tile([128, 1], F32, tag="t1")
                        nc.vector.tensor_tensor(t1[:], s1, e1[:], op=ALU.mult)
                        z = stpool.tile([128, 1], F32, tag="z")
                        nc.vector.scalar_tensor_tensor(z[:], rowsum[:], e2[:], t1[:],
                                                       op0=ALU.mult, op1=ALU.add)
                        rz = stpool.tile([128, 1], F32, tag="rz")
                        nc.vector.reciprocal(rz[:], z[:])
                        r1 = stpool.tile([128, 1], F32, tag="r1")
                        nc.vector.tensor_tensor(r1[:], e1[:], rz[:], op=ALU.mult)
                        r2 = stpool.tile([128, 1], F32, tag="r2")
                        nc.vector.tensor_tensor(r2[:], e2[:], rz[:], op=ALU.mult)
                        for kh in range(2):
                            ob = fbpool.tile([128, 1024], BF16, tag="obf",
                                             name=f"ob{qi}_{kh}")
                            nc.vector.tensor_scalar_mul(
                                ob[:], bb[:, kh * 1024:(kh + 1) * 1024], r2[:])
                            weng = nc.sync if kh == 0 else nc.gpsimd
                            weng.dma_start(
                                out_d[q0:q0 + 128,
                                      KH + kh * 1024:KH + (kh + 1) * 1024],
                                ob[:])
                        # rescale round-0 half from scratch
                        for kh in range(2):
                            fbf = fbpool.tile([128, 1024], BF16, tag="fbf",
                                              name=f"fbf{qi}_{kh}")
                            nc.vector.tensor_scalar_mul(fbf[:], pre_fbb[kh][:],
                                                        r1[:])
                            weng = nc.gpsimd if kh == 0 else nc.sync
                            weng.dma_start(
                                out_d[q0:q0 + 128, kh * 1024:(kh + 1) * 1024],
                                fbf[:])

            # ---- round 0: all proj1 builds first (PE stays fed while the
            # collective + readback complete), then the score blocks ----
            for qc in range(4):
                build_hT(0, qc)
            # builds emitted: free w1/loads/strips/proj-psum, start the peer-iT
            # loads into a fresh pool (disjoint addresses: they only wait on
            # the AllGather, not on round-0 score reads)
            w1ctx.close()
            ldctx.close()
            prctx.close()
            it2pool = it2ctx.enter_context(tc.tile_pool(name="it2", bufs=1))
            it2 = [it2pool.tile([128, KH], F32R, tag=f"jt{mo}", name=f"it1_{mo}")
                   for mo in range(8)]
            for kt in range(4):
                for mo in range(8):
                    nc.sync.dma_start(it2[mo][:, kt * 512:(kt + 1) * 512],
                                      cc_outs[kt][1, mo])
            # round-1 scores get their own 4-buf PSUM pool from the banks the
            # proj pools just freed
            psc2 = it2ctx.enter_context(
                tc.tile_pool(name="psc2", bufs=8, space="PSUM", side="right"))
            for qc in range(4):
                scores_block(0, qc, hTr, iT, None, None)

            # ---- round 1: peer keys ----
            itactx.close()
            fbpool = fbctx.enter_context(
                tc.tile_pool(name="fb", bufs=2, side="right"))
            sc2pool = fbctx.enter_context(
                tc.tile_pool(name="sc2", bufs=3, side="right"))
            for qc in range(4):
                scores_block(1, qc, hTr, it2, fbpool, sc2pool)

            fbctx.close()
            it2ctx.close()

    nc.compile()
    _cache["nc"] = nc
    return nc


def make_in_maps(hidden, pre_emb, w1, b1, w2, b2):
    # biases pre-transposed to [128, 8]: column mo = b[mo*128:(mo+1)*128]
    b1t = np.ascontiguousarray(b1.reshape(8, 128).T)
    b2t = np.ascontiguousarray(b2.reshape(8, 128).T)
    in_maps = []
    for c in range(N_CORES):
        b, p = c // 2, c % 2
        rows = slice(p * QP, (p + 1) * QP)
        in_maps.append({
            "hid_q": np.ascontiguousarray(hidden[b, rows, :]),
            "pre_own": np.ascontiguousarray(pre_emb[b, rows, :]),
            "w1": w1, "w2": w2, "b1": b1t, "b2": b2t,
        })
    return in_maps


def kernel(hidden, pre_emb, w1, b1, w2, b2):
    hidden = np.ascontiguousarray(np.asarray(hidden, dtype=np.float32))
    pre_emb = np.ascontiguousarray(np.asarray(pre_emb, dtype=np.float32))
    w1 = np.ascontiguousarray(np.asarray(w1, dtype=np.float32))
    b1 = np.ascontiguousarray(np.asarray(b1, dtype=np.float32))
    w2 = np.ascontiguousarray(np.asarray(w2, dtype=np.float32))
    b2 = np.ascontiguousarray(np.asarray(b2, dtype=np.float32))

    nc = _build()
    in_maps = make_in_maps(hidden, pre_emb, w1, b1, w2, b2)
    kw = {}
    if TRACE:
        kw = dict(trace=True, trace_cores=[0])
    res = run_bass_kernel_spmd(nc, in_maps, core_ids=list(range(N_CORES)), **kw)
    global LAST_EXEC_NS
    if res.exec_time_ns is not None:
        LAST_EXEC_NS = res.exec_time_ns
    out = np.empty((B, S, S), dtype=np.float32)
    for c in range(N_CORES):
        b, p = c // 2, c % 2
        out[b, p * QP:(p + 1) * QP, :] = np.asarray(
            res.results[c]["out"], dtype=np.float32)
    return out
